# revision 11
# baseline (speedup 1.0000x reference)
"""Trainium2 Bass kernel for CrossMerge3D.

Input ys: [B=2, S=12, C=96, 32, 32, 32] f32. For each (b, c):
  out = (m0 + perm_j(m1) + perm_k(m2)) / 12
where, with the 12 scans split into 3 groups of 4, each group combines as
  m_g = s0 + s1 + flip(s2 + s3)   (flip over the flattened 32^3 volume)
and group 1's volume is stored as (j,k,i), group 2's as (k,i,j); perm_j /
perm_k bring them back to (i,j,k).

Sharding: 8 cores = batch (2) x channel quarters (4) -> 24 channels/core.
No cross-core communication.

Per-core layout: 4 channels x 32 leading-spatial -> 128 SBUF partitions,
1024-wide free dim. All loads are plain mergeable scan-pair DMAs (1 MiB,
fast HWDGE descriptor path; reversed/multi-dim source APs cost ~6.4us
per trigger on the issuing sequencer vs ~0.65us for these). The flip
splits into a free-dim reversal (folded into the pair-sum's operand APs)
and a partition-block reversal (a bit-exact fp32 matmul against a
block-exchange matrix on the otherwise idle TensorEngine). perm_j /
perm_k are DVE 32x32 block transposes plus free-dim permuted APs.
Forward-pair tiles use triple buffering so the DMA pipeline never
drains; loads and stores are spread across both HWDGE rings.
"""

import numpy as np

_B, _S, _C, _D = 2, 12, 96, 32
_NCORE = 8
_CL = _C // 4          # 24 channels per core
_G = _CL // 4          # 6 macro tiles of 4 channels (128 partitions)
_FREE = _D * _D        # 1024

_nc = None


def _build_program():
    from concourse import bacc, tile, mybir

    f32 = mybir.dt.float32
    nc = bacc.Bacc(
        "TRN2", target_bir_lowering=False, debug=False, num_devices=_NCORE
    )
    ys = nc.dram_tensor("ys", [_S, _CL, _D, _D, _D], f32, kind="ExternalInput")
    out = nc.dram_tensor("out", [_CL, _D, _D, _D], f32, kind="ExternalOutput")
    ysa = ys.ap()
    outa = out.ap()

    with tile.TileContext(nc) as tc:
        with (
            tc.tile_pool(name="const", bufs=1) as cst,
            tc.tile_pool(name="io", bufs=2) as iop,
            tc.tile_pool(name="tmp", bufs=2) as tmp,
            tc.tile_pool(name="ps", bufs=1, space="PSUM") as ps,
        ):
            # 32-block exchange stationary (anti-diagonal per block)
            jblk = cst.tile([128, 128], f32, tag="jblk", name="jblk")
            nc.gpsimd.memset(jblk[:], 1.0)
            for b in range(4):
                nc.gpsimd.affine_select(
                    out=jblk[32 * b:32 * b + 32, :],
                    in_=jblk[32 * b:32 * b + 32, :],
                    compare_op=mybir.AluOpType.is_equal, fill=0.0,
                    base=-(32 * b + 31), pattern=[[1, 128]],
                    channel_multiplier=1,
                )

            for g in range(_G):
                cs = slice(4 * g, 4 * (g + 1))

                def load_pair(s, tag, eng, bufs):
                    t = iop.tile([128, 2 * _FREE], f32, tag=tag, name=tag,
                                 bufs=bufs)
                    src = ysa[s:s + 2, cs].rearrange(
                        "s c i j k -> (c i) s (j k)"
                    )
                    dst = t[:].rearrange("p (s f) -> p s f", s=2)
                    eng.dma_start(out=dst, in_=src)
                    return t

                pa = load_pair(0, "pa", nc.sync, 3)
                pr = load_pair(2, "pr", nc.scalar, 2)
                qa = load_pair(4, "qa", nc.sync, 3)
                qr = load_pair(6, "qr", nc.scalar, 2)
                ra = load_pair(8, "ra", nc.sync, 3)
                rr = load_pair(10, "rr", nc.scalar, 2)

                def fwd_sum(t):
                    # in-place into the first half (elementwise aligned)
                    h0, h1 = t[:, 0:_FREE], t[:, _FREE:2 * _FREE]
                    nc.vector.tensor_add(h0, h0, h1)
                    return h0

                def rev_sum(t, tag):
                    # free-dim-reversed pair sum; partition reversal is done
                    # later by the jblk matmul
                    rs = tmp.tile([128, _FREE], f32, tag=tag, name=tag)
                    nc.vector.tensor_add(rs[:], t[:, 0:_FREE][:, ::-1],
                                         t[:, _FREE:2 * _FREE][:, ::-1])
                    return rs

                fA = fwd_sum(pa)
                rA = rev_sum(pr, "rA")
                fB = fwd_sum(qa)
                rB = rev_sum(qr, "rB")
                fC = fwd_sum(ra)
                rC = rev_sum(rr, "rC")

                def flip(rs, name):
                    # partition-block reversal on the TensorEngine
                    pf = ps.tile([128, _FREE], f32, tag="psF", name=name,
                                 bufs=4)
                    for n0 in (0, 512):
                        nc.tensor.matmul(pf[:, n0:n0 + 512], jblk[:],
                                         rs[:][:, n0:n0 + 512],
                                         start=True, stop=True)
                    return pf

                pfA = flip(rA, "pfA")
                pfB = flip(rB, "pfB")
                pfC = flip(rC, "pfC")

                # combines: grp = fwd + flipped_rev (PSUM operand)
                nc.vector.tensor_add(rA[:], fA, pfA[:])
                nc.vector.tensor_add(rB[:], fB, pfB[:])
                nc.vector.tensor_add(rC[:], fC, pfC[:])

                # group 1 ((j,k,i)): 32x32 block transpose, then add with
                # (k,j)->(j,k) free permute
                tb = tmp.tile([128, _FREE], f32, tag="tt", name="tb", bufs=3)
                nc.vector.transpose(tb[:], rB[:])
                acc3 = rA[:].rearrange("p (a b) -> p a b", a=_D)
                tbp = tb[:].rearrange("p (a b) -> p a b", a=_D).transpose(
                    [0, 2, 1]
                )
                nc.vector.tensor_add(acc3, acc3, tbp)

                # group 2 ((k,i,j)): (i,j)->(j,i) free permute (ScalarE),
                # then 32x32 block transpose
                cp = tmp.tile([128, _FREE], f32, tag="cpo", name="cp", bufs=3)
                rcp = rC[:].rearrange("p (a b) -> p a b", a=_D).transpose(
                    [0, 2, 1]
                )
                nc.scalar.copy(cp[:].rearrange("p (a b) -> p a b", a=_D), rcp)
                tcb = tmp.tile([128, _FREE], f32, tag="tt", name="tcb",
                               bufs=3)
                nc.vector.transpose(tcb[:], cp[:])
                nc.vector.tensor_add(rA[:], rA[:], tcb[:])

                o = tmp.tile([128, _FREE], f32, tag="cpo", name="o", bufs=3)
                nc.scalar.mul(o[:], rA[:], 1.0 / 12.0)
                store_eng = nc.sync if g % 2 == 0 else nc.scalar
                store_eng.dma_start(
                    out=outa[cs].rearrange("c i j k -> (c i) (j k)"), in_=o[:]
                )

    nc.compile()
    return nc


def kernel(ys):
    global _nc
    ys = np.ascontiguousarray(ys, dtype=np.float32)
    assert ys.shape == (_B, _S, _C, _D, _D, _D), ys.shape

    if _nc is None:
        _nc = _build_program()

    from concourse.bass_utils import run_bass_kernel_spmd

    in_maps = []
    for r in range(_NCORE):
        b, q = divmod(r, 4)
        shard = np.ascontiguousarray(ys[b, :, q * _CL:(q + 1) * _CL])
        in_maps.append({"ys": shard})

    res = run_bass_kernel_spmd(_nc, in_maps, list(range(_NCORE)))

    out = np.empty((_B, _C, _D, _D, _D), np.float32)
    for r in range(_NCORE):
        b, q = divmod(r, 4)
        out[b, q * _CL:(q + 1) * _CL] = res.results[r]["out"]

    if res.exec_time_ns is not None:
        print(f"HW exec time: {res.exec_time_ns} ns")
    return out
